# revision 23
# baseline (speedup 1.0000x reference)
"""Distributed Trainium2 kernel for a dense-transformer attention block.

Math (matches the reference):
    xqkv = x @ Wqkv + bqkv ; split into q,k,v heads
    scores = (q k^T) / sqrt(HD) + mask ; attn = softmax(scores)
    o = attn @ v ; out = o @ Wproj + bproj

Parallelization over 8 NeuronCores (tensor-parallel over heads):
  - Each core owns NH/8 = 2 heads: computes the QKV projection for its
    heads only (weight-column shard), runs causal attention for those
    heads over the full batch/sequence, then an AllToAll redistributes
    the per-head outputs so each core holds ALL head-dims for 1/8 of the
    (batch*seq) rows.  Each core finishes with the output projection for
    its row block; the host concatenates row blocks.
  - Every core casts x to bf16 locally (duplicated work, but it is pure
    DMA/vector and avoids a 133us AllGather serial head).

Layout notes:
  - Activations are kept transposed ([feature, token]) so the model dim
    lands on SBUF partitions for TensorE contraction; x^T tiles are
    produced with the DMA xbar transpose (needs 2-byte dtype -> bf16).
  - Scores are computed transposed (s^T[kv, q]) so softmax row-sums are
    matmuls with a ones-vector and attn@v consumes p^T directly.
  - Softmax skips max-subtraction (|scores| <= ~8 for this problem, and
    exp() is computed in fp32 out of PSUM).
  - The additive mask is analyzed on the host: fully-masked 512x512
    chunks are skipped, fully-visible chunks run unmasked, and mixed
    chunks get (deduplicated) mask tiles added to the score PSUM.
"""

import hashlib
import numpy as np
import ml_dtypes

B, S, DIM, NH = 4, 2048, 2048, 16
HD = DIM // NH  # 128
NCORES = 8
HPC = NH // NCORES          # heads per core = 2
TOK = B * S                 # 8192 tokens
RPC = TOK // NCORES         # rows (tokens) per core = 1024
CH = 512                    # attention chunk (q and kv)
SUB = 128                   # kv subtile
SCALE = 1.0 / float(np.sqrt(HD))

_BF16 = ml_dtypes.bfloat16

_prog_cache = {}


def _analyze_mask(mask):
    """Build the attention schedule from the additive mask.

    sched[qc] = list of (kc, j, q_lo, mask_id, c_lo, c_hi); mask_id is
    -1 when no mask add is needed for the entry.  Mask tiles are already
    transposed to [kv, q] layout and pre-divided by SCALE.
    """
    m = np.asarray(mask, dtype=np.float32).reshape(S, S)
    NEG = -1e8
    sched = []
    tiles = []
    tile_key = {}
    for qc in range(S // CH):
        ents = []
        for kc in range(S // CH):
            blk = m[qc * CH:(qc + 1) * CH, kc * CH:(kc + 1) * CH]
            if np.all(blk <= NEG):
                continue
            for j in range(CH // SUB):
                sub = blk[:, j * SUB:(j + 1) * SUB]       # [CH q, SUB kv]
                if np.all(sub <= NEG):
                    continue
                vis = ~np.all(sub <= NEG, axis=1)
                q_lo = int(np.argmax(vis))
                q_lo = (q_lo // SUB) * SUB
                if not ents:
                    q_lo = 0  # first entry must initialize full PSUM width
                nzrow = np.any(sub[q_lo:, :] != 0.0, axis=1)
                if nzrow.any():
                    first = q_lo + int(np.argmax(nzrow))
                    last = q_lo + len(nzrow) - int(np.argmax(nzrow[::-1]))
                    c_lo = (first // SUB) * SUB
                    c_hi = min(CH, ((last + SUB - 1) // SUB) * SUB)
                    content = np.ascontiguousarray(
                        (sub[c_lo:c_hi, :].T / SCALE).astype(_BF16))
                    key = (c_hi - c_lo,
                           hashlib.md5(content.tobytes()).hexdigest())
                    if key not in tile_key:
                        tile_key[key] = len(tiles)
                        tiles.append(content)
                    ents.append((kc, j, q_lo, tile_key[key], c_lo, c_hi))
                else:
                    ents.append((kc, j, q_lo, -1, 0, 0))
        assert ents, "a full query chunk is masked out; softmax undefined"
        sched.append(ents)
    n_real = len(tiles)
    widths = [t.shape[1] for t in tiles]
    pack = np.zeros((max(1, n_real), SUB, CH), dtype=_BF16)
    for i, t in enumerate(tiles):
        pack[i, :, :t.shape[1]] = t
    return sched, pack, widths, n_real


def _build_program(sched, n_mask_tiles, mask_widths):
    import concourse.bass as bass
    import concourse.tile as tile
    from concourse import bacc, mybir
    from contextlib import ExitStack

    f32 = mybir.dt.float32
    bf16 = mybir.dt.bfloat16
    AF = mybir.ActivationFunctionType
    ALU = mybir.AluOpType

    nc = bacc.Bacc("TRN2", target_bir_lowering=False, debug=False,
                   num_devices=NCORES)

    x_ext = nc.dram_tensor("x", [TOK, DIM], f32, kind="ExternalInput").ap()
    wqkv_ext = nc.dram_tensor("wqkv", [DIM, 3 * HPC * HD], bf16,
                              kind="ExternalInput").ap()
    bqkv_ext = nc.dram_tensor("bqkv", [3 * HPC * HD, 1], f32,
                              kind="ExternalInput").ap()
    maskt_ext = nc.dram_tensor("maskt", [max(1, n_mask_tiles), SUB, CH], bf16,
                               kind="ExternalInput").ap()
    wproj_ext = nc.dram_tensor("wproj", [DIM, DIM], bf16,
                               kind="ExternalInput").ap()
    bproj_ext = nc.dram_tensor("bproj", [1, DIM], f32,
                               kind="ExternalInput").ap()
    out_ext = nc.dram_tensor("out", [RPC, DIM], bf16,
                             kind="ExternalOutput").ap()

    QKW = 3 * HPC * HD        # 768 projection output dims per core
    NDT = DIM // 128          # 16 contraction tiles
    NSC = S // CH             # 4 s-chunks per batch
    rg = [list(range(NCORES))]

    with tile.TileContext(nc) as tc, ExitStack() as top:
        dram = top.enter_context(tc.tile_pool(name="dram", bufs=1,
                                              space="DRAM"))
        # per-(b, s-chunk) bf16 copies of x (xbar transpose sources)
        xb16 = [[dram.tile([CH, DIM], bf16, name=f"xb16_{b}_{sc}")
                 for sc in range(NSC)] for b in range(B)]
        # AllToAll split in two column (token) halves so the output
        # projection of the first half overlaps the second collective
        a2a_in = [dram.tile([DIM, RPC // 2], bf16, name=f"a2a_in{i}")
                  for i in range(2)]
        a2a_out = [dram.tile([DIM, RPC // 2], bf16, name=f"a2a_out{i}")
                   for i in range(2)]

        const = top.enter_context(tc.tile_pool(name="const", bufs=1))
        ones = const.tile([128, 128], bf16, name="ones", tag="ones")
        nc.any.memset(ones[:], 1.0)
        bqk = []
        for t in range(2 * HPC):
            bt = const.tile([128, 1], f32, name=f"bqk{t}", tag=f"bqk{t}")
            nc.sync.dma_start(out=bt[:], in_=bqkv_ext[t * 128:(t + 1) * 128, :])
            bqk.append(bt)
        vb1 = const.tile([1, HPC * HD], f32, name="vb1", tag="vb1")
        nc.sync.dma_start(
            out=vb1[:],
            in_=bqkv_ext[2 * HPC * HD:3 * HPC * HD, :].rearrange("a b -> b a"))
        vbb = const.tile([128, HPC * HD], f32, name="vbb", tag="vbb")
        nc.gpsimd.partition_broadcast(vbb[:], vb1[:])
        bp1 = const.tile([1, DIM], f32, name="bp1", tag="bp1")
        nc.sync.dma_start(out=bp1[:], in_=bproj_ext[:, :])
        msk = []
        for i in range(n_mask_tiles):
            w = mask_widths[i]
            mt = const.tile([128, w], bf16, name=f"msk{i}", tag=f"msk{i}")
            nc.sync.dma_start(out=mt[:], in_=maskt_ext[i, :, :w])
            msk.append(mt)

        psA = top.enter_context(tc.tile_pool(name="psA", bufs=3, space="PSUM"))
        psV = top.enter_context(tc.tile_pool(name="psV", bufs=1, space="PSUM"))
        psO = top.enter_context(tc.tile_pool(name="psO", bufs=2, space="PSUM"))
        psS = top.enter_context(tc.tile_pool(name="psS", bufs=2, space="PSUM"))

        # persistent qkv storage (bf16)
        qT = [[None] * HPC for _ in range(B)]
        kT = [[None] * HPC for _ in range(B)]
        vS = [[None] * HPC for _ in range(B)]
        frees = []
        for b in range(B):
            for h in range(HPC):
                t1, f1 = tc.tile([128, S], bf16, name=f"qT{b}{h}")
                t2, f2 = tc.tile([128, S], bf16, name=f"kT{b}{h}")
                t3, f3 = tc.tile([128, S], bf16, name=f"vS{b}{h}")
                qT[b][h], kT[b][h], vS[b][h] = t1, t2, t3
                frees += [f1, f2, f3]

        with ExitStack() as p1:
            castp = p1.enter_context(tc.tile_pool(name="cast", bufs=4))

            def cast(b):
                # cast batch b of x to bf16 (DMA + DVE/GpSimd only)
                for sc in range(NSC):
                    for t in range(CH // 128):
                        r0 = b * S + sc * CH + t * 128
                        for cc in range(DIM // 512):
                            xf = castp.tile([128, 512], f32, name="xf",
                                            tag="xf")
                            nc.sync.dma_start(
                                out=xf[:],
                                in_=x_ext[r0:r0 + 128,
                                          cc * 512:(cc + 1) * 512])
                            xb = castp.tile([128, 512], bf16, name="xb",
                                            tag="xb")
                            if cc % 2 == 0:
                                nc.vector.tensor_copy(xb[:], xf[:])
                            else:
                                nc.gpsimd.tensor_copy(xb[:], xf[:])
                            nc.sync.dma_start(
                                out=xb16[b][sc][t * 128:(t + 1) * 128,
                                                cc * 512:(cc + 1) * 512],
                                in_=xb[:])

            wpool = p1.enter_context(tc.tile_pool(name="wq", bufs=1))
            wq = []
            for dt_i in range(NDT):
                wt = wpool.tile([128, QKW], bf16, name=f"wq{dt_i}",
                                tag=f"wq{dt_i}")
                nc.sync.dma_start(
                    out=wt[:], in_=wqkv_ext[dt_i * 128:(dt_i + 1) * 128, :])
                wq.append(wt)
            xtp = p1.enter_context(tc.tile_pool(name="xtp", bufs=2))

            def proj(b):
                for sc in range(NSC):
                    xts = []
                    for dt_i in range(NDT):
                        xt = xtp.tile([128, CH], bf16, name=f"xt{dt_i}",
                                      tag=f"xt{dt_i}")
                        nc.sync.dma_start_transpose(
                            xt[:],
                            xb16[b][sc][:, dt_i * 128:(dt_i + 1) * 128])
                        xts.append(xt)
                    for t in range(2 * HPC):
                        ps = psA.tile([128, CH], f32, name="psqk", tag="A")
                        for dt_i in range(NDT):
                            nc.tensor.matmul(
                                ps[:], wq[dt_i][:, t * 128:(t + 1) * 128],
                                xts[dt_i][:],
                                start=(dt_i == 0), stop=(dt_i == NDT - 1))
                        dst = (qT if t < HPC else kT)[b][t % HPC]
                        nc.vector.tensor_scalar_add(
                            dst[:, sc * CH:(sc + 1) * CH], ps[:], bqk[t][:])
                    for st in range(CH // 128):
                        ps = psV.tile([128, HPC * HD], f32, name="psv", tag="V")
                        for dt_i in range(NDT):
                            nc.tensor.matmul(
                                ps[:],
                                xts[dt_i][:, st * 128:(st + 1) * 128],
                                wq[dt_i][:, 2 * HPC * HD:3 * HPC * HD],
                                start=(dt_i == 0), stop=(dt_i == NDT - 1))
                        gst = sc * (CH // 128) + st
                        for h in range(HPC):
                            nc.vector.scalar_tensor_tensor(
                                out=vS[b][h][:, gst * HD:(gst + 1) * HD],
                                in0=ps[:, h * HD:(h + 1) * HD],
                                scalar=1.0,
                                in1=vbb[:, h * HD:(h + 1) * HD],
                                op0=ALU.mult, op1=ALU.add)

            cast(0)
            for b in range(B):
                proj(b)
                if b + 1 < B:
                    cast(b + 1)

        # ---- Phase 2: attention, two heads interleaved to hide exp ----
        with ExitStack() as p2:
            ptp = p2.enter_context(tc.tile_pool(name="ptp", bufs=6))
            recp = p2.enter_context(tc.tile_pool(name="recp", bufs=2))
            otp = p2.enter_context(tc.tile_pool(name="otp", bufs=3))

            def attn_entry(b, h, qc, ent, ei, last, o_ps, s_sum):
                (kc, j, q_lo, mid, c_lo, c_hi) = ent
                kv0 = kc * CH + j * SUB
                sps = psA.tile([128, CH], f32, name="sps", tag="A")
                nc.tensor.matmul(
                    sps[:, q_lo:CH],
                    kT[b][h][:, kv0:kv0 + SUB],
                    qT[b][h][:, qc * CH + q_lo:(qc + 1) * CH],
                    start=True, stop=True)
                if mid >= 0:
                    nc.vector.tensor_add(
                        sps[:, c_lo:c_hi], sps[:, c_lo:c_hi],
                        msk[mid][:, :c_hi - c_lo])
                pT = ptp.tile([128, CH], bf16, name="pT", tag="pT")
                nc.scalar.activation(
                    pT[:, q_lo:CH], sps[:, q_lo:CH], AF.Exp, scale=SCALE)
                nc.tensor.matmul(
                    s_sum[:, q_lo:CH], ones[:], pT[:, q_lo:CH],
                    start=(ei == 0), stop=(ei == last))
                kvt = kc * (CH // SUB) + j
                nc.tensor.matmul(
                    o_ps[:, q_lo:CH],
                    vS[b][h][:, kvt * HD:(kvt + 1) * HD],
                    pT[:, q_lo:CH],
                    start=(ei == 0), stop=(ei == last))

            def attn_epilogue(b, h, qc, o_ps, s_sum):
                rec = recp.tile([128, CH], f32, name="rec", tag="rec")
                nc.vector.reciprocal(rec[:], s_sum[:])
                oT = otp.tile([128, CH], bf16, name="oT", tag="oT")
                nc.vector.tensor_mul(oT[:], o_ps[:], rec[:])
                dest = 2 * b + qc // 2
                r0 = dest * (HPC * HD) + h * HD
                nc.sync.dma_start(
                    out=a2a_in[qc % 2][r0:r0 + HD, :], in_=oT[:])

            for b in range(B):
                for qc in range(NSC):
                    ents = sched[qc]
                    last = len(ents) - 1
                    o0 = psO.tile([128, CH], f32, name="o_ps0", tag="O")
                    o1 = psO.tile([128, CH], f32, name="o_ps1", tag="O")
                    s0 = psS.tile([128, CH], f32, name="s_sum0", tag="Ssum")
                    s1 = psS.tile([128, CH], f32, name="s_sum1", tag="Ssum")
                    for ei, ent in enumerate(ents):
                        attn_entry(b, 0, qc, ent, ei, last, o0, s0)
                        attn_entry(b, 1, qc, ent, ei, last, o1, s1)
                    attn_epilogue(b, 0, qc, o0, s0)
                    attn_epilogue(b, 1, qc, o1, s1)

        for f in reversed(frees):
            f()

        for i in range(2):
            nc.gpsimd.collective_compute(
                "AllToAll", mybir.AluOpType.bypass, replica_groups=rg,
                ins=[a2a_in[i].opt()], outs=[a2a_out[i].opt()])

        # ---- Phase 4: output projection for my RPC rows ----
        with ExitStack() as p4:
            wpp = p4.enter_context(tc.tile_pool(name="wpp", bufs=1))
            wps = []  # [ot][nchunk] -> [128, CH]
            for ot in range(NDT):
                row = []
                for nchunk in range(DIM // CH):
                    wp = wpp.tile([128, CH], bf16, name=f"wp{ot}_{nchunk}",
                                  tag=f"wp{ot}_{nchunk}")
                    nc.sync.dma_start(
                        out=wp[:],
                        in_=wproj_ext[ot * 128:(ot + 1) * 128,
                                      nchunk * CH:(nchunk + 1) * CH])
                    row.append(wp)
                wps.append(row)
            bcproj = p4.enter_context(tc.tile_pool(name="bcproj", bufs=1))
            bpb = bcproj.tile([128, DIM], f32, name="bpb", tag="bpb")
            nc.gpsimd.partition_broadcast(bpb[:], bp1[:])
            ocp = p4.enter_context(tc.tile_pool(name="ocp", bufs=1))
            resp = p4.enter_context(tc.tile_pool(name="resp", bufs=4))
            for half in range(2):
                oc = []
                for ot in range(NDT):
                    t = ocp.tile([128, RPC // 2], bf16,
                                 name=f"oc{half}_{ot}", tag=f"oc{half}_{ot}")
                    nc.sync.dma_start(
                        out=t[:],
                        in_=a2a_out[half][ot * 128:(ot + 1) * 128, :])
                    oc.append(t)
                for rt in range(RPC // 2 // 128):
                    pss = [psA.tile([128, CH], f32, name=f"pso{n}", tag="A")
                           if n < 3 else
                           psO.tile([128, CH], f32, name=f"pso{n}", tag="O")
                           for n in range(DIM // CH)]
                    for ot in range(NDT):
                        for nchunk in range(DIM // CH):
                            nc.tensor.matmul(
                                pss[nchunk],
                                oc[ot][:, rt * 128:(rt + 1) * 128],
                                wps[ot][nchunk][:],
                                start=(ot == 0), stop=(ot == NDT - 1))
                    grow = half * (RPC // 2) + rt * 128
                    for nchunk in range(DIM // CH):
                        res = resp.tile([128, CH], bf16, name="res",
                                        tag="res")
                        nc.vector.scalar_tensor_tensor(
                            out=res[:], in0=pss[nchunk][:], scalar=1.0,
                            in1=bpb[:, nchunk * CH:(nchunk + 1) * CH],
                            op0=ALU.mult, op1=ALU.add)
                        nc.sync.dma_start(
                            out=out_ext[grow:grow + 128,
                                        nchunk * CH:(nchunk + 1) * CH],
                            in_=res[:])

    nc.compile()
    return nc


def _get_program(sched, n_real, mask_widths):
    key = (str(sched), tuple(mask_widths))
    if key not in _prog_cache:
        _prog_cache[key] = _build_program(sched, n_real, mask_widths)
    return _prog_cache[key]


def kernel(x=None, mask=None, Wqkv=None, bqkv=None, Wproj=None, bproj=None,
           start_pos=0, **_unused):
    from concourse.bass_utils import run_bass_kernel_spmd

    x = np.ascontiguousarray(np.asarray(x, dtype=np.float32).reshape(TOK, DIM))
    mask = np.asarray(mask, dtype=np.float32)
    Wqkv = np.asarray(Wqkv, dtype=np.float32)
    bqkv = np.asarray(bqkv, dtype=np.float32)
    Wproj = np.asarray(Wproj, dtype=np.float32)
    bproj = np.asarray(bproj, dtype=np.float32)

    sched, mask_pack, widths, n_real = _analyze_mask(mask)
    nc = _get_program(sched, n_real, widths)

    wproj_bf = np.ascontiguousarray(Wproj.astype(_BF16))
    bproj2 = np.ascontiguousarray(bproj.reshape(1, DIM))

    in_maps = []
    for c in range(NCORES):
        heads = [HPC * c + i for i in range(HPC)]
        cols = []
        for grp in range(3):  # q, k, v column groups
            for hh in heads:
                c0 = grp * DIM + hh * HD
                cols.append((c0, c0 + HD))
        w_sh = np.concatenate([Wqkv[:, a:b] for a, b in cols], axis=1)
        b_sh = np.concatenate([bqkv[a:b] for a, b in cols])
        in_maps.append({
            "x": x,
            "wqkv": np.ascontiguousarray(w_sh.astype(_BF16)),
            "bqkv": np.ascontiguousarray(b_sh.reshape(-1, 1)),
            "maskt": mask_pack,
            "wproj": wproj_bf,
            "bproj": bproj2,
        })

    import os
    kw = {}
    if os.environ.get("KERNEL_TRACE"):
        kw["trace"] = True
    res = run_bass_kernel_spmd(nc, in_maps, core_ids=list(range(NCORES)), **kw)
    globals()["LAST_RUN"] = res
    if getattr(res, "exec_time_ns", None):
        print(f"HW exec time: {res.exec_time_ns} ns")
    outs = [res.results[c]["out"].astype(np.float32) for c in range(NCORES)]
    full = np.concatenate(outs, axis=0).reshape(B, S, DIM)
    return full
